# revision 1
# baseline (speedup 1.0000x reference)
"""MoE (8 experts, top-2, SwiGLU FFN) Trainium2 Bass kernel, expert-parallel over 8 cores.

Self-contained: builds the Bass/Tile program, shards inputs, runs via
run_bass_kernel_spmd on cores 0-7, and reassembles the full output.

Strategy (core e owns expert e):
  - x is replicated to every core (it arrives as a kernel input on each core's HBM).
  - Gate: each core computes logits for its own T/8 "home" tokens with a
    bf16 hi/lo-split matmul (fp32-grade precision), takes top-2 + softmax,
    then AllGathers the tiny routing table so every core sees all tokens.
  - Routing: each core compacts the token list for ITS expert with
    PE-matmul prefix sums + an indexed scatter (dma_scatter_add) building
    a slot table (token id, A2A return slot).
  - FFN: dma_gather(transpose=True) pulls the selected x rows in transposed
    layout; bf16 matmuls with fp32 PSUM accumulate compute the SwiGLU FFN.
  - Return: unweighted y rows are scattered into a per-home-core-blocked
    send buffer, AllToAll'd back; each home core gathers its two rows per
    token and combines them with the softmax weights (+ b2).
"""

import sys

sys.path.insert(0, "/opt/trn_rl_repo")

import numpy as np
import ml_dtypes

import concourse.bass as bass
import concourse.bacc as bacc
import concourse.mybir as mybir
import concourse.tile as tile

E, TOPK, D, H = 8, 2, 1024, 2048
T = 4096            # total tokens (2*2048)
NCORES = 8
TH = T // NCORES    # home tokens per core = 512
C = 1280            # expert token capacity (max observed 1071; Binom(4096,1/4) 9sigma safe)
CS = 192            # per (expert, home) capacity (max observed 153)
TG = 512            # FFN token group size

BF16 = mybir.dt.bfloat16
F32 = mybir.dt.float32
I16 = mybir.dt.int16
AF = mybir.ActivationFunctionType
OP = mybir.AluOpType

bf16 = ml_dtypes.bfloat16


def build_program(mode="full"):
    nc = bacc.Bacc(
        "TRN2",
        target_bir_lowering=False,
        debug=False,
        enable_asserts=True,
        num_devices=NCORES,
    )

    # ---- per-core inputs ----
    xbf = nc.dram_tensor("xbf", [T, D], BF16, kind="ExternalInput")       # replicated x, bf16
    xthi = nc.dram_tensor("xthi", [D, TH], BF16, kind="ExternalInput")    # own-token x^T hi split
    xtlo = nc.dram_tensor("xtlo", [D, TH], BF16, kind="ExternalInput")    # own-token x^T lo split
    gwhi = nc.dram_tensor("gwhi", [D, E], BF16, kind="ExternalInput")     # gate_w^T hi
    gwlo = nc.dram_tensor("gwlo", [D, E], BF16, kind="ExternalInput")     # gate_w^T lo
    w0 = nc.dram_tensor("w0", [D, H], BF16, kind="ExternalInput")         # expert e W0
    w1 = nc.dram_tensor("w1", [D, H], BF16, kind="ExternalInput")         # expert e W1
    w2 = nc.dram_tensor("w2", [H, D], BF16, kind="ExternalInput")         # expert e W2
    b0d = nc.dram_tensor("b0", [H], F32, kind="ExternalInput")
    b1d = nc.dram_tensor("b1", [H], F32, kind="ExternalInput")
    b2d = nc.dram_tensor("b2", [D], F32, kind="ExternalInput")
    eidd = nc.dram_tensor("eid", [128, 1], F32, kind="ExternalInput")     # core's expert id
    # constants (replicated)
    ltrid = nc.dram_tensor("ltri", [128, 128], BF16, kind="ExternalInput")   # L[k,m] = k<=m
    d127d = nc.dram_tensor("d127", [128, 1], F32, kind="ExternalInput")      # delta at k=127
    ones1d = nc.dram_tensor("ones1", [1, 128], F32, kind="ExternalInput")
    iota8d = nc.dram_tensor("iota8", [128, E], F32, kind="ExternalInput")    # each row 0..7
    iotatd = nc.dram_tensor("iotat", [128, T // 128], F32, kind="ExternalInput")  # [p,g]=128g+p
    homeoffd = nc.dram_tensor("homeoff", [1, T // 128], F32, kind="ExternalInput")  # CS*(g//4)
    glinitd = nc.dram_tensor("glinit", [C, 64], F32, kind="ExternalInput")   # col2 = 8*CS + r
    if mode.startswith("back"):
        gidxh = nc.dram_tensor("gidxh", [128, C // 16], I16, kind="ExternalInput")
        asloth = nc.dram_tensor("asloth", [128, C // 16], I16, kind="ExternalInput")
        ridxh = nc.dram_tensor("ridxh", [128, 2 * TH // 16], I16, kind="ExternalInput")
        routh = nc.dram_tensor("routh", [128, TH // 128, 4], F32, kind="ExternalInput")

    out = nc.dram_tensor("out", [TH, D], F32, kind="ExternalOutput")

    KD = D // 128   # 8
    KH = H // 128   # 16
    GT = T // 128   # 32
    NSEND = E * CS  # 1536 rows in the A2A region

    with tile.TileContext(nc) as tc:
        with (
            tc.tile_pool(name="wpool", bufs=1) as wpool,
            tc.tile_pool(name="xg", bufs=1) as xgpool,
            tc.tile_pool(name="big", bufs=1) as bigpool,
            tc.tile_pool(name="ysb", bufs=1) as ypool_sb,
            tc.tile_pool(name="consts", bufs=1) as consts,
            tc.tile_pool(name="rt", bufs=1) as rt,
            tc.tile_pool(name="work", bufs=2) as work,
            tc.tile_pool(name="ps", bufs=6, space="PSUM") as ps,
            tc.tile_pool(name="dram", bufs=1, space="DRAM") as dram,
        ):
            # ---------- DRAM intermediates ----------
            send = dram.tile([NSEND + C, D], BF16)      # A2A send (+ unique dump rows)
            recv = dram.tile([NSEND, D], BF16)          # A2A recv
            glist = dram.tile([C + T, 64], F32)         # compact slot table (+ dump rows)
            slotd = dram.tile([T], I16)                 # per-token glist slot
            rlistd = dram.tile([2 * TH], I16)           # home-side gather slots
            rout_own = dram.tile([TH, 4], F32)
            rout_all = dram.tile([T, 4], F32)

            # ---------- big constant/weight loads ----------
            w0sb = wpool.tile([128, KD, H], BF16)   # [p, k, h] = w0[128k+p, h]
            w1sb = wpool.tile([128, KD, H], BF16)
            w2sb = wpool.tile([128, KH, D], BF16)
            nc.sync.dma_start(w0sb[:], w0.ap().rearrange("(k p) h -> p k h", p=128))
            nc.sync.dma_start(w1sb[:], w1.ap().rearrange("(k p) h -> p k h", p=128))
            nc.sync.dma_start(w2sb[:], w2.ap().rearrange("(k p) d -> p k d", p=128))

            b0sb = consts.tile([128, KH], F32, tag="b0")
            b1sb = consts.tile([128, KH], F32, tag="b1")
            nc.sync.dma_start(b0sb[:], b0d.ap().rearrange("(h p) -> p h", p=128))
            nc.sync.dma_start(b1sb[:], b1d.ap().rearrange("(h p) -> p h", p=128))
            # b2 broadcast to all partitions via step-0 source AP
            b2bc = consts.tile([128, D], F32, tag="b2bc")
            nc.sync.dma_start(b2bc[:], bass.AP(b2d, 0, [[0, 128], [1, D]]))

            ltri = consts.tile([128, 128], BF16, tag="ltri")
            nc.sync.dma_start(ltri[:], ltrid.ap())
            d127 = consts.tile([128, 1], F32, tag="d127")
            nc.sync.dma_start(d127[:], d127d.ap())
            ones1 = consts.tile([1, 128], F32, tag="ones1")
            nc.sync.dma_start(ones1[:], ones1d.ap())
            iota8 = consts.tile([128, E], F32, tag="iota8")
            nc.sync.dma_start(iota8[:], iota8d.ap())
            iotat = consts.tile([128, GT], F32, tag="iotat")
            nc.sync.dma_start(iotat[:], iotatd.ap())
            homeoff = consts.tile([1, GT], F32, tag="homeoff")
            nc.sync.dma_start(homeoff[:], homeoffd.ap())
            eid = consts.tile([128, 1], F32, tag="eid")
            nc.sync.dma_start(eid[:], eidd.ap())

            # ---------- zero the A2A send region; init glist ----------
            zt = consts.tile([128, 1024], BF16, tag="zeros")
            nc.vector.memset(zt[:], 0.0)
            sendv = send[:NSEND, :].rearrange("(a p) d -> p a d", p=128)  # [128, 12, 1024]
            for a in range(NSEND // 128):
                nc.sync.dma_start(sendv[:, a, :], zt[:])
            nc.sync.dma_start(glist[:C, :], glinitd.ap())  # DRAM->DRAM init

            # ---------- gate ----------
            xhisb = bigpool.tile([128, KD, TH], BF16, tag="bigA")
            xlosb = bigpool.tile([128, KD, TH], BF16, tag="bigB")
            nc.sync.dma_start(xhisb[:], xthi.ap().rearrange("(k p) t -> p k t", p=128))
            nc.sync.dma_start(xlosb[:], xtlo.ap().rearrange("(k p) t -> p k t", p=128))
            gwhisb = consts.tile([128, KD, E], BF16, tag="gwhi")
            gwlosb = consts.tile([128, KD, E], BF16, tag="gwlo")
            nc.sync.dma_start(gwhisb[:], gwhi.ap().rearrange("(k p) e -> p k e", p=128))
            nc.sync.dma_start(gwlosb[:], gwlo.ap().rearrange("(k p) e -> p k e", p=128))

            NCH = TH // 128  # 4 home chunks
            do_front = mode in ("full", "front")
            use_coll = mode != "back_nocoll"
            rout_sb = consts.tile([128, NCH, 4], F32, tag="routsb")
            if mode.startswith("back"):
                nc.sync.dma_start(rout_sb[:], routh.ap())
            if do_front:
                eq1sb = rt.tile([128, NCH, E], F32, tag="eq1sb")
                eq2sb = rt.tile([128, NCH, E], F32, tag="eq2sb")
                for c in range(NCH):
                    lg = ps.tile([128, E], F32, tag="ps")
                    tsl = slice(128 * c, 128 * (c + 1))
                    mmi = 0
                    for xs, gs in ((xhisb, gwhisb), (xhisb, gwlosb), (xlosb, gwhisb)):
                        for k in range(KD):
                            nc.tensor.matmul(
                                lg[:],
                                xs[:, k, tsl],
                                gs[:, k, :],
                                start=(mmi == 0),
                                stop=(mmi == 3 * KD - 1),
                            )
                            mmi += 1
                    # top-2 + softmax
                    m1 = work.tile([128, 1], F32, tag="m1")
                    nc.vector.reduce_max(m1[:], lg[:], axis=mybir.AxisListType.X)
                    eq1 = eq1sb[:, c, :]
                    nc.vector.tensor_scalar(eq1, lg[:], m1[:], None, op0=OP.is_equal)
                    msk = work.tile([128, E], F32, tag="msk")
                    nc.vector.scalar_tensor_tensor(
                        msk[:], eq1, -1e30, lg[:], op0=OP.mult, op1=OP.add
                    )
                    m2 = work.tile([128, 1], F32, tag="m2")
                    nc.vector.reduce_max(m2[:], msk[:], axis=mybir.AxisListType.X)
                    eq2 = eq2sb[:, c, :]
                    nc.vector.tensor_scalar(eq2, msk[:], m2[:], None, op0=OP.is_equal)
                    t1 = work.tile([128, E], F32, tag="t1")
                    nc.vector.tensor_tensor(t1[:], eq1, iota8[:], op=OP.mult)
                    nc.vector.reduce_sum(rout_sb[:, c, 0:1], t1[:], axis=mybir.AxisListType.X)
                    t2 = work.tile([128, E], F32, tag="t2")
                    nc.vector.tensor_tensor(t2[:], eq2, iota8[:], op=OP.mult)
                    nc.vector.reduce_sum(rout_sb[:, c, 1:2], t2[:], axis=mybir.AxisListType.X)
                    dt = work.tile([128, 1], F32, tag="dt")
                    nc.vector.tensor_tensor(dt[:], m2[:], m1[:], op=OP.subtract)
                    # w1 = sigmoid(m1 - m2) = 1/(1+exp(m2-m1))
                    nc.scalar.activation(rout_sb[:, c, 2:3], dt[:], AF.Sigmoid, scale=-1.0)
                    nc.vector.tensor_scalar(
                        rout_sb[:, c, 3:4], rout_sb[:, c, 2:3], -1.0, 1.0,
                        op0=OP.mult, op1=OP.add,
                    )
                nc.sync.dma_start(
                    rout_own[:].rearrange("(c p) f -> p c f", p=128), rout_sb[:]
                )
                nc.gpsimd.collective_compute(
                    "AllGather",
                    OP.bypass,
                    replica_groups=[list(range(NCORES))],
                    ins=[rout_own[:].rearrange("a b -> (a b)")],
                    outs=[rout_all[:].rearrange("a b -> (a b)")],
                )

                # ---------- per-expert routing (over all T tokens) ----------
                rall = rt.tile([128, GT, 4], F32, tag="rall")
                nc.sync.dma_start(rall[:], rout_all[:].rearrange("(g p) f -> p g f", p=128))
                hit1 = rt.tile([128, GT], F32, tag="hit1")
                hit2 = rt.tile([128, GT], F32, tag="hit2")
                nc.vector.tensor_scalar(hit1[:], rall[:, :, 0], eid[:], None, op0=OP.is_equal)
                nc.vector.tensor_scalar(hit2[:], rall[:, :, 1], eid[:], None, op0=OP.is_equal)
                onehot = rt.tile([128, GT], F32, tag="onehot")
                nc.vector.tensor_tensor(onehot[:], hit1[:], hit2[:], op=OP.add)
                onebf = rt.tile([128, GT], BF16, tag="onebf")
                nc.vector.tensor_copy(onebf[:], onehot[:])

                # global inclusive prefix count: per-chunk Ltri matmul + chunk carry
                pchunk = ps.tile([128, GT], F32, tag="ps")
                nc.tensor.matmul(pchunk[:], ltri[:], onebf[:], start=True, stop=True)
                possb = rt.tile([128, GT], F32, tag="possb")
                nc.vector.tensor_copy(possb[:], pchunk[:])
                prow = ps.tile([1, GT], F32, tag="ps")
                nc.tensor.matmul(prow[:], d127[:], possb[:], start=True, stop=True)
                srow = rt.tile([1, GT], F32, tag="srow")
                dummy = rt.tile([1, GT], F32, tag="dummy")
                nc.vector.memset(dummy[:], 0.0)
                nc.vector.tensor_tensor_scan(
                    srow[:], prow[:], dummy[:], 0.0, op0=OP.add, op1=OP.bypass
                )
                erow = rt.tile([1, GT], F32, tag="erow")
                nc.vector.tensor_tensor(erow[:], srow[:], prow[:], op=OP.subtract)
                pbc = ps.tile([128, GT], F32, tag="ps")
                nc.tensor.matmul(pbc[:], ones1[:], erow[:], start=True, stop=True)
                posg = rt.tile([128, GT], F32, tag="posg")
                nc.vector.tensor_tensor(posg[:], possb[:], pbc[:], op=OP.add)

                # A2A send slot (valid where onehot=1):
                #   a2a = CS*(g//4) + (pos_incl - segstart[g//4]) - 1
                segv = rt.tile([1, E], F32, tag="segv")
                erow_v = erow[:].rearrange("p (g f) -> p g f", f=4)
                nc.vector.tensor_copy(segv[:], erow_v[:, :, 0])
                segrow = rt.tile([1, GT], F32, tag="segrow")
                segrow_v = segrow[:].rearrange("p (g f) -> p g f", f=4)
                for r in range(4):
                    nc.vector.tensor_copy(segrow_v[:, :, r], segv[:])
                rowoff = rt.tile([1, GT], F32, tag="rowoff")
                nc.vector.tensor_tensor(rowoff[:], erow[:], segrow[:], op=OP.subtract)
                nc.vector.tensor_tensor(rowoff[:], rowoff[:], homeoff[:], op=OP.add)
                pbc2 = ps.tile([128, GT], F32, tag="ps")
                nc.tensor.matmul(pbc2[:], ones1[:], rowoff[:], start=True, stop=True)
                a2af = rt.tile([128, GT], F32, tag="a2af")
                nc.vector.tensor_tensor(a2af[:], possb[:], pbc2[:], op=OP.add)
                nc.vector.tensor_scalar(a2af[:], a2af[:], -1.0, None, op0=OP.add)
                # glist rows are pre-initialized with col2 = 8*CS + r (unique pad
                # dump slots); scatter ADDS, so ship the delta such that
                # init + delta == a2af for the row this token lands in (r = pos-1).

                # glist slot: selected -> pos-1 ; unselected -> C + t - pos (unique dump)
                sA = rt.tile([128, GT], F32, tag="sA")
                nc.vector.tensor_scalar(sA[:], posg[:], 2.0, -(1.0 + C), op0=OP.mult, op1=OP.add)
                nc.vector.tensor_tensor(sA[:], sA[:], iotat[:], op=OP.subtract)
                nc.vector.tensor_tensor(sA[:], sA[:], onehot[:], op=OP.mult)
                sB = rt.tile([128, GT], F32, tag="sB")
                nc.vector.tensor_tensor(sB[:], iotat[:], posg[:], op=OP.subtract)
                nc.vector.tensor_scalar(sB[:], sB[:], float(C), None, op0=OP.add)
                slotf = rt.tile([128, GT], F32, tag="slotf")
                nc.vector.tensor_tensor(slotf[:], sA[:], sB[:], op=OP.add)
                sloti = rt.tile([128, GT], I16, tag="sloti")
                nc.vector.tensor_copy(sloti[:], slotf[:])
                # bounce to DRAM in token order, reload wrapped-16 (x8 replicated)
                nc.sync.dma_start(bass.AP(slotd[:].tensor, 0, [[1, 128], [128, GT]]), sloti[:])
                scat_idx = rt.tile([128, T // 16], I16, tag="scatidx")
                for q in range(8):
                    nc.sync.dma_start(
                        scat_idx[16 * q : 16 * (q + 1), :],
                        bass.AP(slotd[:].tensor, 0, [[1, 16], [16, T // 16]]),
                    )
                # scatter source: col0 = token id, col2 = a2a slot
                scat_in = bigpool.tile([128, GT, 64], F32, tag="bigC")
                nc.vector.memset(scat_in[:], 0.0)
                nc.vector.tensor_copy(scat_in[:, :, 0], iotat[:])
                a2adj = rt.tile([128, GT], F32, tag="a2adj")
                nc.vector.tensor_tensor(a2adj[:], a2af[:], posg[:], op=OP.subtract)
                nc.vector.tensor_scalar(
                    a2adj[:], a2adj[:], float(1 - E * CS), None, op0=OP.add
                )
                nc.vector.tensor_copy(scat_in[:, :, 2], a2adj[:])
                nc.gpsimd.dma_scatter_add(
                    glist[:],
                    scat_in[:],
                    scat_idx[:],
                    num_idxs=T,
                    num_idxs_reg=T,
                    elem_size=64,
                )

                # reload compacted lists: gidx (token ids), aslot (a2a slots)
                gidxf = rt.tile([128, C // 16], F32, tag="gidxf")
                aslotf = rt.tile([128, C // 16], F32, tag="aslotf")
                for q in range(8):
                    nc.sync.dma_start(
                        gidxf[16 * q : 16 * (q + 1), :],
                        bass.AP(glist[:].tensor, 0, [[64, 16], [64 * 16, C // 16]]),
                    )
                    nc.sync.dma_start(
                        aslotf[16 * q : 16 * (q + 1), :],
                        bass.AP(glist[:].tensor, 2, [[64, 16], [64 * 16, C // 16]]),
                    )
                gidx = rt.tile([128, C // 16], I16, tag="gidx")
                nc.vector.tensor_copy(gidx[:], gidxf[:])
                aslot = rt.tile([128, C // 16], I16, tag="aslot")
                nc.vector.tensor_copy(aslot[:], aslotf[:])

            else:
                gidx = rt.tile([128, C // 16], I16, tag="gidx")
                nc.sync.dma_start(gidx[:], gidxh.ap())
                aslot = rt.tile([128, C // 16], I16, tag="aslot")
                nc.sync.dma_start(aslot[:], asloth.ap())
            # ---------- gather x rows (transposed), <=512 idxs per call ----------
            xtgs = {}
            o = 0
            while o < C:
                gl = min(TG, C - o)
                xt = xgpool.tile([128, KD, gl], BF16, tag=f"xtg{o}")
                nc.gpsimd.dma_gather(
                    xt[:], xbf.ap(), gidx[:, o // 16 : (o + gl) // 16],
                    num_idxs=gl, num_idxs_reg=gl, elem_size=D, transpose=True,
                )
                xtgs[o] = xt
                o += TG

            if do_front and mode == "full":
                # ---------- home-side gather slots (before FFN; only needs own gate) ----------
                # per-expert inclusive counts over my TH tokens, chunk-chained
                posi8 = rt.tile([128, NCH, E], F32, tag="posi8")
                carry = rt.tile([1, E], F32, tag="carry")
                nc.vector.memset(carry[:], 0.0)
                oh8 = rt.tile([128, NCH, E], BF16, tag="oh8")
                nc.vector.tensor_tensor(oh8[:], eq1sb[:], eq2sb[:], op=OP.add)
                for c in range(NCH):
                    psI = ps.tile([128, E], F32, tag="ps")
                    nc.tensor.matmul(psI[:], ltri[:], oh8[:, c, :], start=True, stop=True)
                    psC = ps.tile([128, E], F32, tag="ps")
                    nc.tensor.matmul(psC[:], ones1[:], carry[:], start=True, stop=True)
                    nc.vector.tensor_copy(posi8[:, c, :], psI[:])
                    nc.vector.tensor_tensor(posi8[:, c, :], posi8[:, c, :], psC[:], op=OP.add)
                    if c < NCH - 1:
                        prow8 = ps.tile([1, E], F32, tag="ps")
                        nc.tensor.matmul(prow8[:], d127[:], posi8[:, c, :], start=True, stop=True)
                        nc.vector.tensor_copy(carry[:], prow8[:])
                # r1/r2 slots: CS*sel + (pos at sel) - 1
                r12 = rt.tile([128, E], F32, tag="r12")
                for c in range(NCH):
                    pm = work.tile([128, E], F32, tag="pm")
                    nc.vector.tensor_tensor(pm[:], posi8[:, c, :], eq1sb[:, c, :], op=OP.mult)
                    p1 = work.tile([128, 1], F32, tag="p1")
                    nc.vector.reduce_sum(p1[:], pm[:], axis=mybir.AxisListType.X)
                    t5 = work.tile([128, 1], F32, tag="t5")
                    nc.vector.tensor_scalar(
                        t5[:], rout_sb[:, c, 0:1], float(CS), -1.0, op0=OP.mult, op1=OP.add
                    )
                    nc.vector.tensor_tensor(r12[:, c : c + 1], t5[:], p1[:], op=OP.add)
                    nc.vector.tensor_tensor(pm[:], posi8[:, c, :], eq2sb[:, c, :], op=OP.mult)
                    nc.vector.reduce_sum(p1[:], pm[:], axis=mybir.AxisListType.X)
                    nc.vector.tensor_scalar(
                        t5[:], rout_sb[:, c, 1:2], float(CS), -1.0, op0=OP.mult, op1=OP.add
                    )
                    nc.vector.tensor_tensor(r12[:, NCH + c : NCH + c + 1], t5[:], p1[:], op=OP.add)
                r12i = rt.tile([128, E], I16, tag="r12i")
                nc.vector.tensor_copy(r12i[:], r12[:])
                nc.sync.dma_start(bass.AP(rlistd[:].tensor, 0, [[1, 128], [128, E]]), r12i[:])
                ridx = rt.tile([128, 2 * TH // 16], I16, tag="ridx")
                for q in range(8):
                    nc.sync.dma_start(
                        ridx[16 * q : 16 * (q + 1), :],
                        bass.AP(rlistd[:].tensor, 0, [[1, 16], [16, 2 * TH // 16]]),
                    )

            elif mode.startswith("back"):
                ridx = rt.tile([128, 2 * TH // 16], I16, tag="ridx")
                nc.sync.dma_start(ridx[:], ridxh.ap())
            if mode == "front":
                dump = work.tile([128, D], F32, tag="oc")
                nc.vector.tensor_copy(dump[:], xtg[:, :, 0:512].rearrange("p k t -> p (k t)")[:, :D])
                outv0 = out.ap().rearrange("(c p) d -> p c d", p=128)
                for c in range(TH // 128):
                    nc.sync.dma_start(outv0[:, c, :], dump[:])
            else:
                # ---------- FFN ----------
                groups = []
                o = 0
                while o < C:
                    groups.append((o, min(TG, C - o)))
                    o += TG
                for goff, glen in groups:
                    ysb = ypool_sb.tile([128, TG // 128, D], BF16, tag="ysbg")
                    gt = bigpool.tile([128, KH, glen], BF16, tag="bigA")
                    for h in range(KH):
                        ph1 = ps.tile([128, glen], F32, tag="ps")
                        for k in range(KD):
                            nc.tensor.matmul(
                                ph1[:],
                                w1sb[:, k, 128 * h : 128 * (h + 1)],
                                xtgs[goff][:, k, :],
                                start=(k == 0),
                                stop=(k == KD - 1),
                            )
                        ph0 = ps.tile([128, glen], F32, tag="ps")
                        for k in range(KD):
                            nc.tensor.matmul(
                                ph0[:],
                                w0sb[:, k, 128 * h : 128 * (h + 1)],
                                xtgs[goff][:, k, :],
                                start=(k == 0),
                                stop=(k == KD - 1),
                            )
                        # silu(z) = z * sigmoid(z), z = h1 + b1  (Silu not in CoreSim)
                        sig = work.tile([128, TG], F32, tag="sig")
                        nc.scalar.activation(
                            sig[:, :glen], ph1[:], AF.Sigmoid, bias=b1sb[:, h : h + 1]
                        )
                        zb = work.tile([128, TG], F32, tag="zb")
                        nc.vector.tensor_scalar(
                            zb[:, :glen], ph1[:], b1sb[:, h : h + 1], None, op0=OP.add
                        )
                        nc.vector.tensor_tensor(
                            zb[:, :glen], zb[:, :glen], sig[:, :glen], op=OP.mult
                        )
                        # gT = (h0 + b0) * silu
                        nc.vector.scalar_tensor_tensor(
                            gt[:, h, :],
                            ph0[:],
                            b0sb[:, h : h + 1],
                            zb[:, :glen],
                            op0=OP.add,
                            op1=OP.mult,
                        )
                    nch = glen // 128
                    for c in range(nch):
                        for n in range(D // 512):
                            py = ps.tile([128, 512], F32, tag="ps")
                            for k in range(KH):
                                nc.tensor.matmul(
                                    py[:],
                                    gt[:, k, 128 * c : 128 * (c + 1)],
                                    w2sb[:, k, 512 * n : 512 * (n + 1)],
                                    start=(k == 0),
                                    stop=(k == KH - 1),
                                )
                            nc.vector.tensor_copy(ysb[:, c, 512 * n : 512 * (n + 1)], py[:])
                    # scatter this group's rows into the A2A send buffer
                    nc.gpsimd.dma_scatter_add(
                        send[:],
                        ysb[:, :nch, :],
                        aslot[:, goff // 16 : (goff + glen) // 16],
                        num_idxs=glen,
                        num_idxs_reg=glen,
                        elem_size=D,
                    )

                # ---------- return A2A + home combine ----------
                if use_coll:
                    nc.gpsimd.collective_compute(
                        "AllToAll",
                        OP.bypass,
                        replica_groups=[list(range(NCORES))],
                        ins=[send[:NSEND, :].rearrange("a b -> (a b)")],
                        outs=[recv[:].rearrange("a b -> (a b)")],
                    )
                else:
                    nc.sync.dma_start(recv[:], send[:NSEND, :])
                y12 = bigpool.tile([128, 2 * TH // 128, D], BF16, tag="bigA")
                for b in range(2):
                    nc.gpsimd.dma_gather(
                        y12[:, 4 * b : 4 * (b + 1), :], recv[:].opt(),
                        ridx[:, 32 * b : 32 * (b + 1)],
                        num_idxs=TH, num_idxs_reg=TH,
                        elem_size=D, transpose=False,
                    )
                outv = out.ap().rearrange("(c p) d -> p c d", p=128)
                for c in range(NCH):
                    oc = work.tile([128, D], F32, tag="oc")
                    nc.vector.tensor_scalar(
                        oc[:], y12[:, c, :], rout_sb[:, c, 2:3], None, op0=OP.mult
                    )
                    nc.vector.scalar_tensor_tensor(
                        oc[:], y12[:, NCH + c, :], rout_sb[:, c, 3:4], oc[:],
                        op0=OP.mult, op1=OP.add,
                    )
                    nc.vector.tensor_tensor(oc[:], oc[:], b2bc[:], op=OP.add)
                    nc.sync.dma_start(outv[:, c, :], oc[:])

    nc.compile()
    return nc


def _split_bf16(a):
    hi = a.astype(bf16)
    lo = (a - hi.astype(np.float32)).astype(bf16)
    return hi, lo


def make_in_maps(inputs, gate_w, W0, b0, W1, b1, W2, b2):
    x = np.ascontiguousarray(np.asarray(inputs).reshape(-1, D).astype(np.float32))
    xbf = x.astype(bf16)
    gwT = np.ascontiguousarray(np.asarray(gate_w).astype(np.float32).T)  # [D, E]
    gwhi, gwlo = _split_bf16(gwT)

    ltri = np.ascontiguousarray(np.triu(np.ones((128, 128), np.float32))).astype(bf16)
    d127 = np.zeros((128, 1), np.float32)
    d127[127, 0] = 1.0
    ones1 = np.ones((1, 128), np.float32)
    iota8 = np.tile(np.arange(E, dtype=np.float32)[None, :], (128, 1))
    iotat = np.ascontiguousarray(
        np.arange(T, dtype=np.float32).reshape(T // 128, 128).T
    )
    homeoff = (CS * (np.arange(T // 128, dtype=np.float32) // 4))[None, :]
    glinit = np.zeros((C, 64), np.float32)
    glinit[:, 2] = E * CS + np.arange(C, dtype=np.float32)

    W0 = np.asarray(W0)
    W1 = np.asarray(W1)
    W2 = np.asarray(W2)
    b0 = np.asarray(b0)
    b1 = np.asarray(b1)
    b2 = np.asarray(b2)

    in_maps = []
    for e in range(NCORES):
        xT_own = np.ascontiguousarray(x[e * TH : (e + 1) * TH].T)  # [D, TH]
        xthi, xtlo = _split_bf16(xT_own)
        m = {
            "xbf": xbf,
            "xthi": xthi,
            "xtlo": xtlo,
            "gwhi": gwhi,
            "gwlo": gwlo,
            "w0": np.ascontiguousarray(W0[e].astype(bf16)),
            "w1": np.ascontiguousarray(W1[e].astype(bf16)),
            "w2": np.ascontiguousarray(W2[e].astype(bf16)),
            "b0": np.ascontiguousarray(b0[e].astype(np.float32)),
            "b1": np.ascontiguousarray(b1[e].astype(np.float32)),
            "b2": np.ascontiguousarray(b2[e].astype(np.float32)),
            "eid": np.full((128, 1), float(e), np.float32),
            "ltri": ltri,
            "d127": d127,
            "ones1": ones1,
            "iota8": iota8,
            "iotat": iotat,
            "homeoff": homeoff,
            "glinit": glinit,
        }
        in_maps.append(m)
    return in_maps


_NC_CACHE = {}


def get_program(mode="full"):
    import os
    mode = os.environ.get("KERNEL_MODE", mode)
    if mode not in _NC_CACHE:
        _NC_CACHE[mode] = build_program(mode)
    return _NC_CACHE[mode]


def kernel(**inputs):
    from concourse.bass_utils import run_bass_kernel_spmd

    nc = get_program()
    in_maps = make_in_maps(**inputs)
    res = run_bass_kernel_spmd(nc, in_maps, core_ids=list(range(NCORES)))
    outs = [np.asarray(res.results[c]["out"], dtype=np.float32) for c in range(NCORES)]
    full = np.concatenate(outs, axis=0)
    return full.reshape(np.asarray(inputs["inputs"]).shape)

